# revision 1
# baseline (speedup 1.0000x reference)
"""Trainium2 Bass kernel for nn_NetworkBasic (2-layer SLAYER SNN).

Per core (batch sharded 2/core across 8 cores):
  stage A (TensorE): temporal matmul  mid = dataT^T @ T.  dataT is the 0/1
      spike tensor pre-transposed to [(t,w2) partitions, (b,wp,h) free];
      layer 0's dataT comes from the HOST (free), layer 1's from PE
      transposes of s1.  T is an interleaved block-diag [2T,2T] f16 hi+lo
      pair (2 accumulating matmuls per chunk).  mid is evacuated as an
      f16 hi+lo pair (exact to ~2^-22).
  stage B (TensorE): spatial 3x3 conv as banded-H matmuls with w-shifted
      PSUM accumulation, 3 cross terms (Hhi*mhi + Hhi*mlo + Hlo*mhi) in
      f16 (full PE rate), split into t-halves so the scan can start after
      the first half while the second half computes.
  scan (VectorE): 2nd-order membrane recurrence in t-major layout
      (all operands contiguous), 2 ops/time-step:
      y_t     = (m[t] <= th) + 2d*m[t] + What[t+1]      (custom DVE op)
      m[t+1]  = -d^2 * m[t-1] + y_t                     (scalar_tensor_tensor)
  spikes (GpSimd): s = (m <= th) -> f16, per t-slab, overlapped under the
      scan.

I/O: f16 both ways; host does the transpose/cast glue (exact 0/1 values).

Membrane math: the refractory alpha kernel ref[k] = A*k*d^k is realized as
an IIR via scaled variables (scale c = 1/(A*d) < 0, which flips >= to <=).
The FIR truncation tail of the reference is ~1e-4 and is ignored.
"""

import os
import numpy as np

import concourse.bass as bass
import concourse.mybir as mybir
from concourse import bacc, bass_utils
from concourse.tile import TileContext
from concourse.masks import make_identity

F32 = mybir.dt.float32
F16 = mybir.dt.float16
AO = mybir.AluOpType

# ---------------- problem constants (hardcoded) ----------------
B_FULL, H, W, T = 16, 128, 64, 64
N_CORES = 8
B_LOC = B_FULL // N_CORES          # 2
BW = B_LOC * W                     # 128 (b,w) columns per core
SP_FREE = BW * T                   # 8192 free elements ([128, 8192] tensors)
NPAIR = B_LOC * W // 2             # 64 transposed (b,w-pair) chunks
WG = 8                             # stage-B w-group size
NWG = W // WG                      # 8 w-groups
TH = T // 2                        # stage-B t-half size
NSLAB = 4                          # spike-threshold slabs
SLAB_T = T // NSLAB

THETA = (30.0, 50.0)
TAU_SR = (1.0, 2.0)
TAU_REF = (1.0, 2.0)

WARM_PE = os.environ.get("KERNEL_WARM_PE", "0") == "1"


def _alpha_kernel(tau, mult, eps):
    vals = []
    for t in np.arange(0.0, float(T), 1.0):
        v = mult * t / tau * np.exp(1.0 - t / tau)
        if abs(v) < eps and t > tau:
            break
        vals.append(v)
    if len(vals) < 2:
        vals.append(0.0)
    return np.asarray(vals, np.float32)


SRM_K = [_alpha_kernel(TAU_SR[i], 1.0, 0.01) for i in range(2)]


def _layer_consts(layer):
    d = float(np.exp(-1.0 / TAU_REF[layer]))
    A = -2.0 * THETA[layer] * np.e / TAU_REF[layer]   # ref[k] = A*k*d^k
    c = 1.0 / (A * d)
    theta_hat = float(np.float32(c * THETA[layer]))
    return d, theta_hat


def _temporal_mat(layer):
    """[64,64] fp64 matrix:  what[t'] = sum_t data[t] * M[t, t']."""
    d, _ = _layer_consts(layer)
    A = -2.0 * THETA[layer] * np.e / TAU_REF[layer]
    c = 1.0 / (A * d)
    kern = SRM_K[layer].astype(np.float64)
    P = np.zeros((T, T))
    for t in range(T):
        for k in range(len(kern)):
            if t + k < T:
                P[t, t + k] = kern[k]
    D = np.zeros((T, T))
    for t in range(T):
        D[t, t] = 1.0
        if t + 1 < T:
            D[t, t + 1] = -2.0 * d
        if t + 2 < T:
            D[t, t + 2] = d * d
    return c * (P @ D)


def _hilo_f16(M):
    hi = M.astype(np.float16)
    lo = (M.astype(np.float32) - hi.astype(np.float32)).astype(np.float16)
    return hi, lo


def _hilo_f16_blockdiag(M):
    """Block-diag over w2: Tb[w2*T+t, w2*T+t'] = M[t, t']."""
    hi, lo = _hilo_f16(M)
    bhi = np.zeros((2 * T, 2 * T), np.float16)
    blo = np.zeros((2 * T, 2 * T), np.float16)
    for i in (0, 1):
        bhi[i * T:(i + 1) * T, i * T:(i + 1) * T] = hi
        blo[i * T:(i + 1) * T, i * T:(i + 1) * T] = lo
    return bhi, blo


def _h_mats(w, which):
    """w: [3,3] fp -> [3,128,128]; Hm[dwi][h, hp] = w[h-hp+1, dwi]."""
    out = np.zeros((3, H, H), np.float16)
    for dwi in range(3):
        for dh in (-1, 0, 1):
            v = np.float16(w[dh + 1, dwi]) if which == "hi" else np.float16(
                np.float32(w[dh + 1, dwi]) - np.float32(np.float16(w[dh + 1, dwi])))
            for hp in range(H):
                h = hp + dh
                if 0 <= h < H:
                    out[dwi, h, hp] = v
    return out


# ---------------- custom DVE op registration ----------------
_SNN_OP = None


def _register_snn_op():
    global _SNN_OP
    if _SNN_OP is not None:
        return _SNN_OP
    import concourse.dve_ops as dve_ops
    from concourse.dve_spec import Spec, Src0, Src1, C0, C1, lower
    from concourse.dve_uop import DveOpSpec

    name = "SNN_STEP_ANT"
    if name in dve_ops._SUB_OPCODE_FOR_NAME:
        _SNN_OP = next(op for op in dve_ops.OPS if op.name == name)
        return _SNN_OP

    # out = (s0 >= in0) + in0*s1 + in1
    body = (C0 >= Src0) + Src0 * C1 + Src1
    spec = Spec(
        body=body,
        reference=lambda in0, in1, s0, s1, imm2: (
            (np.float32(s0) >= in0).astype(np.float32)
            + in0 * np.float32(s1)
            + in1
        ).astype(np.float32),
    )
    row = 1 + len(dve_ops.OPS)
    shas = {}
    for ver in ("v3", "v4"):
        try:
            tmp = DveOpSpec(name=name, opcode=row, uops=lower(spec, ver=ver), rd1_en=True)
            shas[ver] = tmp.sha(ver)
        except Exception:
            pass
    op = dve_ops.DveOp(name, spec, subdim=False, uops_sha=shas)
    dve_ops.OPS.append(op)
    dve_ops._SUB_OPCODE_FOR_NAME[name] = row
    dve_ops.CUSTOM_DVE_SPECS[name] = spec
    _SNN_OP = op
    return op


# ---------------- bass kernel trace ----------------
def trace_kernel(nc, xt_d, t_d, h_d, out_d):
    snn_op = _register_snn_op()
    G = NPAIR // 4       # 16 stage-A groups of 4 chunks

    with TileContext(nc) as tc:
        with (
            tc.tile_pool(name="const", bufs=1) as cpool,
            tc.tile_pool(name="big", bufs=1) as bpool,
            tc.tile_pool(name="scan", bufs=2) as ypool,
            tc.tile_pool(name="ptrans", bufs=2, space="PSUM") as pt_pool,
            tc.tile_pool(name="pa", bufs=2, space="PSUM") as pa_pool,
            tc.tile_pool(name="pb", bufs=2, space="PSUM") as pb_pool,
        ):
            # constants
            ident = cpool.tile([H, H], F16)
            make_identity(nc, ident)
            tmats = {}
            for layer in (0, 1):
                thi = cpool.tile([2 * T, 2 * T], F16, tag=f"thi{layer}")
                tlo = cpool.tile([2 * T, 2 * T], F16, tag=f"tlo{layer}")
                nc.sync.dma_start(out=thi, in_=t_d[layer][0].ap())
                nc.sync.dma_start(out=tlo, in_=t_d[layer][1].ap())
                tmats[layer] = (thi, tlo)
            hmats = {}
            for layer in (0, 1):
                hmh = cpool.tile([H, 3 * H], F16, tag=f"hh{layer}")
                hml = cpool.tile([H, 3 * H], F16, tag=f"hl{layer}")
                for tile_, dram in ((hmh, h_d[layer][0]), (hml, h_d[layer][1])):
                    nc.sync.dma_start(
                        out=tile_[:, :].rearrange("p (k n) -> p k n", k=3),
                        in_=dram.ap().rearrange("k p n -> p k n"),
                    )
                hmats[layer] = (hmh, hml)

            # input: pre-transposed f16, DMA in 4 chunks for overlap with A
            dataT0 = bpool.tile([H, SP_FREE], F16, tag="dataT")
            for q in range(4):
                sl = slice(q * 2048, (q + 1) * 2048)
                nc.sync.dma_start(out=dataT0[:, sl], in_=xt_d.ap()[:, sl])

            dataT = dataT0
            for layer in (0, 1):
                d, theta_hat = _layer_consts(layer)
                thi, tlo = tmats[layer]
                hmh, hml = hmats[layer]

                # ---- stage A: temporal matmuls + f16 hi/lo evacuation ----
                midh = bpool.tile([H, SP_FREE], F16, tag="midh")
                midl = bpool.tile([H, SP_FREE], F16, tag="midl")
                scopeA = nc.enter_named_scope(f"stageA{layer}", False)
                for g in range(G):
                    pa = pa_pool.tile([H, 4 * H], F32, tag="pa")
                    for c2 in range(4):
                        chunk = g * 4 + c2
                        lhsT = dataT[:, chunk * H:(chunk + 1) * H]
                        nc.tensor.matmul(
                            pa[:, c2 * H:(c2 + 1) * H], lhsT, thi,
                            start=True, stop=False, skip_group_check=True,
                        )
                        nc.tensor.matmul(
                            pa[:, c2 * H:(c2 + 1) * H], lhsT, tlo,
                            start=False, stop=True, skip_group_check=True,
                        )
                    sl = slice(g * 512, (g + 1) * 512)
                    nc.scalar.copy(midh[:, sl], pa)
                    # midl = pa - midh  (psum src + sbuf src), f16 out
                    nc.vector.scalar_tensor_tensor(
                        midl[:, sl], midh[:, sl], -1.0, pa, AO.mult, AO.add,
                    )
                nc.leave_named_scope(f"stageA{layer}", scopeA[0], False)

                # ---- stage B: spatial conv, f16 3-term, by (t-half, b, wg) ----
                # what layout: t-major [p, (t, b, w)]
                what = bpool.tile([H, SP_FREE], F32, tag="what")
                mvh = midh[:, :].rearrange("p (b w t) -> p b w t", b=B_LOC, w=W)
                mvl = midl[:, :].rearrange("p (b w t) -> p b w t", b=B_LOC, w=W)
                whatv = what[:, :].rearrange("p (t b w) -> p t b w", t=T, b=B_LOC)
                scopeB = nc.enter_named_scope(f"stageB{layer}", False)
                nev = 0
                for th in range(2):
                    ts = slice(th * TH, (th + 1) * TH)
                    for b in range(B_LOC):
                        for wg in range(NWG):
                            w0 = wg * WG
                            pb = pb_pool.tile([H, WG * TH], F32, tag="pb")
                            pbv = pb[:, :].rearrange("p (w t) -> p w t", w=WG)
                            first = True
                            for hm_, mv_ in ((hmh, mvh), (hmh, mvl), (hml, mvh)):
                                # center band
                                nc.tensor.matmul(
                                    pbv[:, :, :], hm_[:, H:2 * H],
                                    mv_[:, b, w0:w0 + WG, ts],
                                    start=first, stop=False, skip_group_check=True,
                                )
                                first = False
                                # left neighbor: out[w] += H_L @ mid[w-1]
                                if wg == 0:
                                    nc.tensor.matmul(
                                        pbv[:, 1:, :], hm_[:, 0:H],
                                        mv_[:, b, 0:WG - 1, ts],
                                        start=False, stop=False, skip_group_check=True,
                                    )
                                else:
                                    nc.tensor.matmul(
                                        pbv[:, :, :], hm_[:, 0:H],
                                        mv_[:, b, w0 - 1:w0 + WG - 1, ts],
                                        start=False, stop=False, skip_group_check=True,
                                    )
                                # right neighbor: out[w] += H_R @ mid[w+1]
                                last = hm_ is hml
                                if wg == NWG - 1:
                                    nc.tensor.matmul(
                                        pbv[:, :WG - 1, :], hm_[:, 2 * H:3 * H],
                                        mv_[:, b, w0 + 1:w0 + WG, ts],
                                        start=False, stop=last, skip_group_check=True,
                                    )
                                else:
                                    nc.tensor.matmul(
                                        pbv[:, :, :], hm_[:, 2 * H:3 * H],
                                        mv_[:, b, w0 + 1:w0 + WG + 1, ts],
                                        start=False, stop=last, skip_group_check=True,
                                    )
                            dst = whatv[:, ts, b, w0:w0 + WG]
                            src = pb[:, :].rearrange("p (w t) -> p t w", w=WG)
                            if nev % 2 == 0:
                                nc.scalar.copy(dst, src)
                            else:
                                nc.vector.tensor_copy(dst, src)
                            nev += 1
                nc.leave_named_scope(f"stageB{layer}", scopeB[0], False)

                # ---- scan (t-major: every operand contiguous) ----
                scopeS = nc.enter_named_scope(f"scan{layer}", False)
                mh = bpool.tile([H, SP_FREE], F32, tag=f"mh{layer}")

                def msl(t):
                    return mh[:, t * BW:(t + 1) * BW]

                def wsl(t):
                    return what[:, t * BW:(t + 1) * BW]

                spk = bpool.tile([H, SP_FREE], F16,
                                 tag="dataT" if layer == 0 else "s2")
                # layer 0: (b,w,t)-major (for contiguous PE transposes);
                # layer 1: t-major (contiguous thresholds + slab DMA out)
                spkv = spk[:, :].rearrange(
                    "p (b w t) -> p b w t", b=B_LOC, w=W)
                mhv = mh[:, :].rearrange(
                    "p (t b w) -> p b w t", t=T, b=B_LOC)
                nc.scalar.copy(msl(0), wsl(0))
                two_d = float(np.float32(2.0 * d))
                md2 = float(np.float32(-(d * d)))
                warm = []
                for t in range(T - 1):
                    if t == 0:
                        nc.vector._custom_dve(
                            snn_op, out=msl(1), in0=msl(0),
                            in1=wsl(1), s0=theta_hat, s1=two_d,
                        )
                    else:
                        y = ypool.tile([H, BW], F32, tag="y")
                        nc.vector._custom_dve(
                            snn_op, out=y, in0=msl(t),
                            in1=wsl(t + 1), s0=theta_hat, s1=two_d,
                        )
                        nc.vector.scalar_tensor_tensor(
                            msl(t + 1), msl(t - 1), md2, y,
                            AO.mult, AO.add,
                        )
                    # keep the PE HAM warm during the long scan with a tiny
                    # transpose that depends on the just-written scan step,
                    # so the PE wakes every ~3us instead of idling cold
                    if WARM_PE and t % 3 == 2 and t < T - 4:
                        pw = pt_pool.tile([H, H], F16, tag="warm")
                        nc.tensor.transpose(
                            pw, msl(t).bitcast(F16)[:, :H], ident)
                        warm.append(pw)
                    # layer-0: early-threshold t in [48,60) once m[59] lands
                    if layer == 0 and t == 58:
                        nc.vector.tensor_scalar(
                            spkv[:, :, :, T - SLAB_T:T - 4],
                            mhv[:, :, :, T - SLAB_T:T - 4],
                            theta_hat, None, AO.is_le)
                    # threshold finished t-slabs
                    if (t + 1) % SLAB_T == 0 and (t + 1) < T:
                        t0s = t + 1 - SLAB_T
                        if layer == 0:
                            nc.vector.tensor_scalar(
                                spkv[:, :, :, t0s:t + 1],
                                mhv[:, :, :, t0s:t + 1],
                                theta_hat, None, AO.is_le)
                        elif False:
                            pass
                        else:
                            sl = slice(t0s * BW, (t + 1) * BW)
                            nc.vector.tensor_scalar(
                                spk[:, sl], mh[:, sl], theta_hat, None,
                                AO.is_le)
                            nc.sync.dma_start(
                                out=out_d.ap()[:, sl], in_=spk[:, sl])
                nc.leave_named_scope(f"scan{layer}", scopeS[0], False)
                # last slab: mostly pre-thresholded at t=59, remainder here
                if layer == 0:
                    nc.vector.tensor_scalar(
                        spkv[:, :, :, T - 4:T], mhv[:, :, :, T - 4:T],
                        theta_hat, None, AO.is_le)
                else:
                    sl = slice((T - SLAB_T) * BW, T * BW)
                    nc.vector.tensor_scalar(
                        spk[:, sl], mh[:, sl], theta_hat, None, AO.is_le)
                    nc.sync.dma_start(out=out_d.ap()[:, sl], in_=spk[:, sl])

                if layer == 0:
                    # transpose s1 chunks on PE -> next layer's dataT
                    dataT = bpool.tile([H, SP_FREE], F16, tag="dataT2")
                    scopeT = nc.enter_named_scope("trans1", False)
                    for g in range(G):
                        ptr = pt_pool.tile([H, 4 * H], F16, tag="ptr")
                        for c2 in range(4):
                            chunk = g * 4 + c2
                            nc.tensor.transpose(
                                ptr[:, c2 * H:(c2 + 1) * H],
                                spk[:, chunk * H:(chunk + 1) * H],
                                ident,
                            )
                        sl = slice(g * 512, (g + 1) * 512)
                        if g % 2 == 0:
                            nc.scalar.copy(dataT[:, sl], ptr)
                        else:
                            nc.vector.tensor_copy(dataT[:, sl], ptr)
                    nc.leave_named_scope("trans1", scopeT[0], False)
    return nc


_BUILT = {}


def _build():
    global _BUILT
    key = (WARM_PE,)
    if key in _BUILT:
        return _BUILT[key]
    nc = bacc.Bacc("TRN2", debug=False)
    xt_d = nc.dram_tensor("xt", [H, SP_FREE], F16, kind="ExternalInput")
    t_d, h_d = {}, {}
    for layer in (0, 1):
        t_d[layer] = (
            nc.dram_tensor(f"t{layer}hi", [2 * T, 2 * T], F16, kind="ExternalInput"),
            nc.dram_tensor(f"t{layer}lo", [2 * T, 2 * T], F16, kind="ExternalInput"),
        )
        h_d[layer] = (
            nc.dram_tensor(f"h{layer}hi", [3, H, H], F16, kind="ExternalInput"),
            nc.dram_tensor(f"h{layer}lo", [3, H, H], F16, kind="ExternalInput"),
        )
    out_d = nc.dram_tensor("out", [H, SP_FREE], F16, kind="ExternalOutput")
    trace_kernel(nc, xt_d, t_d, h_d, out_d)
    nc.compile()
    _BUILT[key] = nc
    return nc


def _host_inputs(conv1_w, conv2_w):
    ins = {}
    for layer, w in ((0, conv1_w), (1, conv2_w)):
        hi, lo = _hilo_f16_blockdiag(_temporal_mat(layer))
        ins[f"t{layer}hi"] = hi
        ins[f"t{layer}lo"] = lo
        w2 = np.asarray(w, np.float32).reshape(3, 3)
        ins[f"h{layer}hi"] = _h_mats(w2, "hi")
        ins[f"h{layer}lo"] = _h_mats(w2, "lo")
    return ins


def make_in_maps(spikeInput, conv1_w, conv2_w):
    x = np.asarray(spikeInput, np.float32).reshape(B_FULL, H, W, T)
    x16 = x.astype(np.float16)                      # exact: values are 0/1
    common = _host_inputs(conv1_w, conv2_w)
    in_maps = []
    for c in range(N_CORES):
        xc = x16[c * B_LOC:(c + 1) * B_LOC]         # [b, h, w, t]
        xc = xc.reshape(B_LOC, H, W // 2, 2, T)     # b h wp w2 t
        xt = np.ascontiguousarray(xc.transpose(3, 4, 0, 2, 1))  # w2 t b wp h
        m = dict(common)
        m["xt"] = xt.reshape(H, SP_FREE)
        in_maps.append(m)
    return in_maps


def kernel(spikeInput, conv1_w, conv2_w):
    nc = _build()
    in_maps = make_in_maps(spikeInput, conv1_w, conv2_w)
    res = bass_utils.run_bass_kernel_spmd(nc, in_maps, core_ids=list(range(N_CORES)))
    outs = []
    for r in res.results:
        o = r["out"].reshape(H, T, B_LOC, W)        # h t b w
        outs.append(o.transpose(2, 0, 3, 1))        # b h w t
    return np.concatenate(outs, axis=0).astype(np.float32)

